# revision 31
# baseline (speedup 1.0000x reference)
"""Trainium2 Bass kernel for the NeuralODE problem.

Math (matching reference.py):
    20 Euler steps (10 segments x 2 steps, uniform dt => step size hi = 0.05):
        z_{i+1} = z_i + hi * ( tanh(z_i @ W1 + b1 + t_i*wt) @ W2 + b2 )

Shared device-side reformulation (per core, batch shard B=64):
    - Fold hi into W2:  W2' = hi * W2, c = hi * b2.
    - Keep the "state without accumulated c":  z'_i = z_i - i*c, so
        z'_{i+1} = z'_i + tanh(z'_i @ W1 + bias_i) @ W2'
      with bias_i = b1 + t_i*wt + i*(c @ W1)   (precomputed on host).
      Final output: z_20 = z'_20 + 20*c       (added on host).
    - State kept transposed (d-major) as zT[p, 64k+b] = z'[b, 128k+p].

Variant "b" (default): bf16 weights-STATIONARY scheme, ~3x the fp32 variant.
    - Both matmuls keep a [128,128] weight block as the stationary operand and
      stream the 64-wide batch as the moving operand (bf16 => 1 cyc/row, full
      128x128 PE utilization, LDWEIGHTS hides under the previous matmul).
    - mm1 output lands hid-major (bias is a per-partition vector there), mm2's
      moving operand is the tanh output directly, and mm2 output lands d-major
      = the state layout. No PE transposes at all.
    - PSUM discipline: a start=True matmul logically zeroes its whole 2KB bank
      ("zero region"), so accumulation groups in one bank must be strictly
      sequential and bank reuse must wait for all readers. mm1 groups rotate
      over 4 banks (region hm//4) with the last 4 groups in 2 spare banks;
      mm2 over 2 banks. Violating this loses accumulated partial sums.
    - Bias enters via a coarse DVE add (pre-broadcast bf16 bias tile) per bank
      plus one coarse tanh per bank; the last 4 groups use fused tanh+bias so
      the mm1->mm2 transition tail is one short activation.
    - State accumulates in fp32 (DVE add from PSUM, deferred past mm2); a bf16
      shadow (zbf = bf16(zt + f), DVE, per d-tile) is the mm1 moving operand
      and unblocks the next step early (numerics: ~1.4e-3 final rel err).
    - Measured on 8 trn2 cores: ~233-278 us vs 851 us for variant a.

Variant "a": fp32 batch-stationary scheme (512-wide weight streams, PE
    transposes between the two matmuls). Exact but ~4x slower: fp32 matmuls
    cost 4 cyc/row, and fp32r is rejected by the ISA for the tile_position
    PSUM packing this layout needs (s3d3_mm_valid_dst_partition).

Sharding: pure data-parallel over batch (512 -> 8 x 64); weights replicated.
"""

import numpy as np

BS, D, HID = 512, 1024, 2048
NCORES = 8
B = BS // NCORES  # 64
NSTEP = 20
KD = D // 128  # 8 k-tiles for the D contraction
KH = HID // 128  # 16 k-tiles for the HID contraction
F32 = np.float32

VARIANT = "b"  # "b" (bf16 weights-stationary) or "a" (fp32 batch-stationary)
MM_DTYPE = "float32"  # variant a: "float32" or "float32r" (f32r fails ISA check)


# --------------------------------------------------------------------------
# Variant B: bf16 weights-stationary, no transposes
# --------------------------------------------------------------------------


def _build_program_b():
    import concourse.mybir as mybir
    from concourse import bacc
    from concourse.tile import TileContext

    nc = bacc.Bacc()
    f32 = mybir.dt.float32
    bf16 = mybir.dt.bfloat16
    TANH = mybir.ActivationFunctionType.Tanh
    COPY = mybir.ActivationFunctionType.Copy

    zt_in = nc.dram_tensor("zt_in", [128, KD * B], f32, kind="ExternalInput")
    zbf_in = nc.dram_tensor("zbf_in", [128, KD * B], bf16, kind="ExternalInput")
    w1_d = nc.dram_tensor("w1", [128, KD * HID], bf16, kind="ExternalInput")
    w2_d = nc.dram_tensor("w2", [128, KH * D], bf16, kind="ExternalInput")
    # biases packed per-partition: biases_d[p, i*KH + m] = bias_i[m*128 + p]
    biases_d = nc.dram_tensor("biases", [128, NSTEP * KH], f32, kind="ExternalInput")
    # bias values pre-broadcast over the batch for the coarse (per-bank) adds:
    # bias_tiled[p, i*768 + b*192 + r*64 + c] = bias_i[(4r+b)*128 + p], r<3
    biast_d = nc.dram_tensor("bias_tiled", [128, NSTEP * 768], bf16, kind="ExternalInput")
    zt_out = nc.dram_tensor("zt_out", [128, KD * B], f32, kind="ExternalOutput")

    with (
        TileContext(nc) as tc,
        tc.tile_pool(name="weights", bufs=1) as wpool,
        tc.tile_pool(name="state", bufs=1) as spool,
        tc.tile_pool(name="hbuf", bufs=2) as hpool,
        tc.tile_pool(name="psumh", bufs=1, space="PSUM") as ph_pool,
        tc.tile_pool(name="psumf", bufs=1, space="PSUM") as pf_pool,
    ):
        zt = spool.tile([128, KD * B], f32, tag="zt")
        zbf = spool.tile([128, KD * B], bf16, tag="zbf")
        bias_sb = wpool.tile([128, NSTEP * KH], f32, tag="bias")
        biast = wpool.tile([128, NSTEP * 768], bf16, tag="biast")

        # Weights live in per-group blocks (w1: one block per hm with all its
        # k-slices; w2: one block per dm with all its q-slices) so step 0's
        # hm-outer groups stream straight off the DMA arrival order.
        w1b = [
            wpool.tile([128, KD * 128], bf16, tag=f"w1_{hm}", name="w1b")
            for hm in range(KH)
        ]
        w2b = [
            wpool.tile([128, KH * 128], bf16, tag=f"w2_{dm}", name="w2b")
            for dm in range(KD)
        ]
        # DMA issues cost ~0.6us each on their issuing engine's sequencer;
        # spread them over three queues so step 0 isn't issue-serialized.
        nc.sync.dma_start(zbf[:], zbf_in[:])
        nc.scalar.dma_start(w1b[0][:], w1_d[:, 0 : KD * 128])
        nc.gpsimd.dma_start(bias_sb[:], biases_d[:])
        nc.gpsimd.dma_start(biast[:, 0 : 4 * 768], biast_d[:, 0 : 4 * 768])
        for hm in range(1, KH):
            eng = nc.sync if hm % 2 == 0 else nc.scalar
            eng.dma_start(w1b[hm][:], w1_d[:, hm * KD * 128 : (hm + 1) * KD * 128])
        nc.gpsimd.dma_start(zt[:], zt_in[:])
        for dm in range(KD):
            eng = nc.sync if dm % 2 == 0 else nc.scalar
            eng.dma_start(w2b[dm][:], w2_d[:, dm * KH * 128 : (dm + 1) * KH * 128])
        nc.sync.dma_start(biast[:, 4 * 768 :], biast_d[:, 4 * 768 :])

        for i in range(NSTEP):
            # ---- mm1: hT[hm] = bias_i[hm] + sum_k W1[k,hm]^T zbf[k] ----
            # PSUM rule: a start in a bank waits for stop of the bank's prior
            # group and all its readers (2KB zero region). Groups rotate over
            # 4 banks (hm%4, region hm//4 for hm<12); the last four groups
            # live in two spare banks so their starts don't WAR against the
            # coarse tanh reads of banks 0-3.
            ph = [
                ph_pool.tile(
                    [128, 3 * B], f32, tag=f"ph{b}", name="ph",
                    padded_shape=[128, 8 * B],
                )
                for b in range(4)
            ]
            # phx banks host the last four mm1 groups (regions 0-1) and are
            # reused for half the mm2 groups (regions 2-3) once the fused
            # tanhs have consumed them.
            phx = [
                ph_pool.tile(
                    [128, 4 * B], f32, tag=f"phx{b}", name="phx",
                    padded_shape=[128, 8 * B],
                )
                for b in range(2)
            ]
            h_bf = hpool.tile([128, KH * B], bf16, tag="hbf")

            def ph_ap(hm):
                if hm >= 12:
                    e = hm - 12
                    return phx[e % 2][:, (e // 2) * B : (e // 2) * B + B]
                return ph[hm // 3][:, (hm % 3) * B : (hm % 3) * B + B]
            for hm in range(KH):
                for k in range(KD):
                    nc.tensor.matmul(
                        ph_ap(hm),
                        w1b[hm][:, k * 128 : k * 128 + 128],
                        zbf[:, k * B : k * B + B],
                        start=(k == 0),
                        stop=(k == KD - 1),
                    )
                if hm >= 12:
                    # last four groups: fused tanh(x + bias) per region so the
                    # tail after the final group is one short activation
                    nc.scalar.activation(
                        h_bf[:, hm * B : hm * B + B],
                        ph_ap(hm),
                        TANH,
                        bias=bias_sb[:, i * KH + hm : i * KH + hm + 1],
                    )
                elif hm % 3 == 2:
                    # bank b=hm//3 holds the three consecutive groups 3b..3b+2
                    # and just completed: coarse DVE bias-add then one coarse
                    # tanh over [128, 192]. Early banks finish 3 groups in, so
                    # h production trails mm1 by only ~3 groups.
                    b = hm // 3
                    nc.vector.tensor_add(
                        ph[b][:, 0 : 3 * B],
                        ph[b][:, 0 : 3 * B],
                        biast[:, i * 768 + b * 192 : i * 768 + b * 192 + 192],
                    )
                    nc.scalar.activation(
                        h_bf[:, 3 * b * B : (3 * b + 3) * B],
                        ph[b][:, 0 : 3 * B],
                        TANH,
                    )

            # ---- mm2: f[dm] = sum_q W2'[q,dm]^T h[q]; d-major PSUM ----
            # Spread the 8 dm groups over FOUR tiles (2 pf banks + the 2 phx
            # banks): a start=True matmul conflicts with its whole tile, so a
            # tile's next start waits on the previous group's adds; 4 tiles
            # give that chain a 3-group window instead of 1.
            pf = [
                pf_pool.tile(
                    [128, 2 * B], f32, tag=f"pf{b}", name="pf",
                    padded_shape=[128, 8 * B],
                )
                for b in range(2)
            ]

            def pf_ap(dm):
                t = dm % 4
                r = dm // 4
                if t < 2:
                    return pf[t][:, r * B : r * B + B]
                return phx[t - 2][:, (2 + r) * B : (2 + r) * B + B]

            # plain ascending q-order already matches h availability (coarse
            # tanh of bank b yields chunks 3b..3b+2 in order, then fused 12-15)
            for dm in range(KD):
                for qi, q in enumerate(range(KH)):
                    nc.tensor.matmul(
                        pf_ap(dm),
                        w2b[dm][:, q * 128 : q * 128 + 128],
                        h_bf[:, q * B : q * B + B],
                        start=(qi == 0),
                        stop=(qi == KH - 1),
                    )
                # bf16 shadow per-dm (critical path: feeds next step's mm1;
                # dead on the final step)
                if i < NSTEP - 1:
                    nc.vector.tensor_add(
                        zbf[:, dm * B : dm * B + B],
                        zt[:, dm * B : dm * B + B],
                        pf_ap(dm),
                    )
                else:
                    nc.vector.tensor_add(
                        zt[:, dm * B : dm * B + B],
                        zt[:, dm * B : dm * B + B],
                        pf_ap(dm),
                    )
                    if dm == 3:
                        nc.sync.dma_start(
                            zt_out[:, 0 : 4 * B], zt[:, 0 : 4 * B]
                        )
            # fp32 state updates deferred past mm2: they read the pf banks,
            # and the pf banks' next starts are a full step away, while the
            # mm2 phase keeps DVE traffic off the banks the PE accumulates.
            # (On the final step the zt-adds ran inline above instead, so the
            # kernel tail is just the output DMA.)
            if i < NSTEP - 1:
                for dm in range(KD):
                    nc.vector.tensor_add(
                        zt[:, dm * B : dm * B + B],
                        zt[:, dm * B : dm * B + B],
                        pf_ap(dm),
                    )

        nc.sync.dma_start(zt_out[:, 4 * B :], zt[:, 4 * B :])

    nc.compile()
    return nc


# --------------------------------------------------------------------------
# Variant A: fp32 batch-stationary (original baseline)
# --------------------------------------------------------------------------


def _build_program_a(mm_dtype=MM_DTYPE, repeat=1):
    import concourse.mybir as mybir
    from concourse import bacc
    from concourse.tile import TileContext

    nc = bacc.Bacc()
    f32 = mybir.dt.float32
    mmdt = getattr(mybir.dt, mm_dtype)
    TANH = mybir.ActivationFunctionType.Tanh

    zt_in = nc.dram_tensor("zt_in", [128, KD * B], mmdt, kind="ExternalInput")
    w1_d = nc.dram_tensor("w1", [128, KD * HID], mmdt, kind="ExternalInput")
    w2_d = nc.dram_tensor("w2", [128, KH * D], mmdt, kind="ExternalInput")
    biases_d = nc.dram_tensor("biases", [NSTEP, HID], mmdt, kind="ExternalInput")
    ident_d = nc.dram_tensor("ident", [128, 128], mmdt, kind="ExternalInput")
    ones_d = nc.dram_tensor("ones", [1, B], mmdt, kind="ExternalInput")
    zt_out = nc.dram_tensor("zt_out", [128, KD * B], mmdt, kind="ExternalOutput")

    with (
        TileContext(nc) as tc,
        tc.tile_pool(name="const", bufs=1) as cpool,
        tc.tile_pool(name="weights", bufs=1) as wpool,
        tc.tile_pool(name="state", bufs=1) as spool,
        tc.tile_pool(name="work", bufs=2) as hpool,
        tc.tile_pool(name="bias", bufs=2) as bpool,
        tc.tile_pool(name="psumh", bufs=2, space="PSUM") as ph_pool,
        tc.tile_pool(name="psumt", bufs=2, space="PSUM") as pt_pool,
        tc.tile_pool(name="psumf", bufs=2, space="PSUM") as pf_pool,
    ):
        ident_sb = cpool.tile([128, 128], mmdt, tag="ident")
        nc.sync.dma_start(ident_sb[:], ident_d[:])
        ones_sb = cpool.tile([1, B], mmdt, tag="ones")
        nc.sync.dma_start(ones_sb[:], ones_d[:])

        zt = spool.tile([128, KD * B], mmdt, tag="zt")
        nc.sync.dma_start(zt[:], zt_in[:])
        hT = spool.tile([128, KH * B], mmdt, tag="hT")

        w1t = []
        for k in range(KD):
            w = wpool.tile([128, HID], mmdt, tag=f"w1_{k}")
            nc.sync.dma_start(w[:], w1_d[:, k * HID : (k + 1) * HID])
            w1t.append(w)
        w2t = []
        for k in range(KH):
            w = wpool.tile([128, D], mmdt, tag=f"w2_{k}")
            nc.sync.dma_start(w[:], w2_d[:, k * D : (k + 1) * D])
            w2t.append(w)

        def scan_body(_iv=None):
            for i in range(NSTEP):
                bias_sb = bpool.tile([1, HID], mmdt, tag="bias")
                nc.sync.dma_start(bias_sb[:], biases_d[i : i + 1, :])

                phs = []
                for g in range(2):
                    ph = ph_pool.tile([128, 512], f32, tag="ph")
                    phs.append(ph)
                    for half in range(2):
                        c = 2 * g + half
                        nc.tensor.matmul(
                            ph[64 * half : 64 * half + 64, :],
                            ones_sb[:1, :],
                            bias_sb[:1, 512 * c : 512 * c + 512],
                            start=True,
                            stop=False,
                            tile_position=(0, 64 * half),
                        )
                    for k in range(KD):
                        for half in range(2):
                            c = 2 * g + half
                            nc.tensor.matmul(
                                ph[64 * half : 64 * half + 64, :],
                                zt[:, B * k : B * k + B],
                                w1t[k][:, 512 * c : 512 * c + 512],
                                start=False,
                                stop=(k == KD - 1),
                                tile_position=(0, 64 * half),
                            )

                for g in range(2):
                    h_bm = hpool.tile([128, 512], mmdt, tag="h_bm")
                    nc.scalar.activation(h_bm[:], phs[g][:], TANH)
                    pt = pt_pool.tile([128, 512], mmdt, tag="pt")
                    for u in range(4):
                        nc.tensor.matmul(
                            pt[:, 128 * u : 128 * u + 128],
                            h_bm[:, 128 * u : 128 * u + 128],
                            ident_sb[:],
                            is_transpose=True,
                            start=True,
                            stop=True,
                        )
                    nc.vector.tensor_copy(
                        hT[:, 512 * g : 512 * g + 512].rearrange(
                            "p (h u c) -> p h u c", h=2, u=4
                        ),
                        pt[:].rearrange("p (u h c) -> p h u c", u=4, h=2),
                    )

                pf = pf_pool.tile([128, 512], f32, tag="pf")
                for k in range(KH):
                    for half in range(2):
                        nc.tensor.matmul(
                            pf[64 * half : 64 * half + 64, :],
                            hT[:, B * k : B * k + B],
                            w2t[k][:, 512 * half : 512 * half + 512],
                            start=(k == 0),
                            stop=(k == KH - 1),
                            tile_position=(0, 64 * half),
                        )

                f_bm = hpool.tile([128, 512], mmdt, tag="f_bm")
                nc.vector.tensor_copy(f_bm[:], pf[:])
                pt2 = pt_pool.tile([128, 512], mmdt, tag="pt")
                for u in range(4):
                    nc.tensor.matmul(
                        pt2[:, 128 * u : 128 * u + 128],
                        f_bm[:, 128 * u : 128 * u + 128],
                        ident_sb[:],
                        is_transpose=True,
                        start=True,
                        stop=True,
                    )
                zt_v = zt[:].rearrange("p (h u c) -> p h u c", h=2, u=4)
                nc.vector.tensor_add(
                    zt_v, zt_v, pt2[:].rearrange("p (u h c) -> p h u c", u=4, h=2)
                )

        if repeat == 1:
            scan_body()
        else:
            with tc.For_i(0, repeat, 1) as _i:
                scan_body(_i)

        nc.sync.dma_start(zt_out[:, 4 * B :], zt[:, 4 * B :])

    nc.compile()
    return nc


# --------------------------------------------------------------------------
# Host-side packing
# --------------------------------------------------------------------------


def _pack_zT(shard):  # [B, D] -> [128, KD*B]
    return np.ascontiguousarray(
        shard.T.reshape(KD, 128, B).transpose(1, 0, 2).reshape(128, KD * B)
    )


def _unpack_zT(zt):  # [128, KD*B] -> [B, D]
    return zt.reshape(128, KD, B).transpose(1, 0, 2).reshape(D, B).T


def _host_common(z0, t, W1, b1, wt, W2, b2):
    t = np.asarray(t, F32)
    t0s, t1s = t[:-1], t[1:]
    h_seg = (t1s - t0s) / 2.0  # N_STEPS_PER_SEG = 2
    step_ts = (
        t0s[:, None] + h_seg[:, None] * np.arange(2, dtype=F32)[None, :]
    ).reshape(-1)
    step_hs = np.repeat(h_seg, 2)
    assert np.allclose(step_hs, step_hs[0]), "non-uniform Euler steps unsupported"
    scale = F32(step_hs[0])

    c = (scale * np.asarray(b2, F32)).astype(F32)  # [D]
    cW1 = (c.astype(np.float64) @ np.asarray(W1, np.float64)).astype(F32)  # [HID]
    biases = np.stack(
        [
            (np.asarray(b1, F32) + step_ts[i] * np.asarray(wt, F32) + i * cW1).astype(
                F32
            )
            for i in range(NSTEP)
        ]
    )  # [NSTEP, HID]
    return biases, scale, c


def _make_in_maps_b(z0, t, W1, b1, wt, W2, b2):
    import ml_dtypes

    bf16 = ml_dtypes.bfloat16
    z0 = np.asarray(z0, F32)
    biases, scale, c = _host_common(z0, t, W1, b1, wt, W2, b2)

    bias_cols = np.ascontiguousarray(
        biases.reshape(NSTEP, KH, 128).transpose(2, 0, 1).reshape(128, NSTEP * KH)
    )
    # bias_tiled[p, i*768 + b*192 + r*64 + c] = biases[i, (3b+r)*128 + p], r<3
    A = biases.reshape(NSTEP, KH, 128)[:, :12, :].reshape(NSTEP, 4, 3, 128)
    bias_tiled = np.ascontiguousarray(
        np.broadcast_to(
            A.transpose(3, 0, 1, 2)[:, :, :, :, None], (128, NSTEP, 4, 3, B)
        ).reshape(128, NSTEP * 768)
    ).astype(bf16)
    # w1p[p, hm*KD*128 + k*128 + c] = W1[k*128+p, hm*128+c]
    w1p = np.ascontiguousarray(
        np.asarray(W1, F32)
        .reshape(KD, 128, KH, 128)
        .transpose(1, 2, 0, 3)
        .reshape(128, KD * HID)
    ).astype(bf16)
    # w2p[p, dm*KH*128 + q*128 + c] = W2'[q*128+p, dm*128+c]
    w2p = np.ascontiguousarray(
        (scale * np.asarray(W2, F32))
        .astype(F32)
        .reshape(KH, 128, KD, 128)
        .transpose(1, 2, 0, 3)
        .reshape(128, KH * D)
    ).astype(bf16)

    in_maps = []
    for core in range(NCORES):
        shard = z0[core * B : (core + 1) * B]
        ztp = _pack_zT(shard)
        in_maps.append(
            {
                "zt_in": ztp,
                "zbf_in": ztp.astype(bf16),
                "w1": w1p,
                "w2": w2p,
                "biases": bias_cols,
                "bias_tiled": bias_tiled,
            }
        )
    return in_maps, c


def _make_in_maps_a(z0, t, W1, b1, wt, W2, b2):
    z0 = np.asarray(z0, F32)
    biases, scale, c = _host_common(z0, t, W1, b1, wt, W2, b2)
    w1p = np.ascontiguousarray(
        np.asarray(W1, F32)
        .reshape(KD, 128, HID)
        .transpose(1, 0, 2)
        .reshape(128, KD * HID)
    )
    w2p = np.ascontiguousarray(
        (scale * np.asarray(W2, F32))
        .astype(F32)
        .reshape(KH, 128, D)
        .transpose(1, 0, 2)
        .reshape(128, KH * D)
    )
    ident = np.eye(128, dtype=F32)
    ones = np.ones((1, B), F32)
    in_maps = []
    for core in range(NCORES):
        shard = z0[core * B : (core + 1) * B]
        in_maps.append(
            {
                "zt_in": _pack_zT(shard),
                "w1": w1p,
                "w2": w2p,
                "biases": biases,
                "ident": ident,
                "ones": ones,
            }
        )
    return in_maps, c


def run(z0, t, W1, b1, wt, W2, b2, trace=False, mm_dtype=MM_DTYPE, variant=VARIANT):
    from concourse.bass_utils import run_bass_kernel_spmd

    if variant == "b":
        in_maps, c = _make_in_maps_b(z0, t, W1, b1, wt, W2, b2)
        nc = _build_program_b()
    else:
        in_maps, c = _make_in_maps_a(z0, t, W1, b1, wt, W2, b2)
        nc = _build_program_a(mm_dtype=mm_dtype)
    res = run_bass_kernel_spmd(nc, in_maps, core_ids=list(range(NCORES)), trace=trace)

    outs = []
    for core in range(NCORES):
        z_shard = _unpack_zT(np.asarray(res.results[core]["zt_out"], F32))
        outs.append(z_shard)
    out = np.concatenate(outs, axis=0).astype(F32)
    out = out + (NSTEP * c)[None, :].astype(F32)
    return out.astype(F32), res


def kernel(z0, t, W1, b1, wt, W2, b2):
    out, _ = run(z0, t, W1, b1, wt, W2, b2, trace=False)
    return out


# revision 32
# speedup vs baseline: 1.0147x; 1.0147x over previous
"""Trainium2 Bass kernel for the NeuralODE problem.

Math (matching reference.py):
    20 Euler steps (10 segments x 2 steps, uniform dt => step size hi = 0.05):
        z_{i+1} = z_i + hi * ( tanh(z_i @ W1 + b1 + t_i*wt) @ W2 + b2 )

Shared device-side reformulation (per core, batch shard B=64):
    - Fold hi into W2:  W2' = hi * W2, c = hi * b2.
    - Keep the "state without accumulated c":  z'_i = z_i - i*c, so
        z'_{i+1} = z'_i + tanh(z'_i @ W1 + bias_i) @ W2'
      with bias_i = b1 + t_i*wt + i*(c @ W1)   (precomputed on host).
      Final output: z_20 = z'_20 + 20*c       (added on host).
    - State kept transposed (d-major) as zT[p, 64k+b] = z'[b, 128k+p].

Variant "b" (default): bf16 weights-STATIONARY scheme, ~3x the fp32 variant.
    - Both matmuls keep a [128,128] weight block as the stationary operand and
      stream the 64-wide batch as the moving operand (bf16 => 1 cyc/row, full
      128x128 PE utilization, LDWEIGHTS hides under the previous matmul).
    - mm1 output lands hid-major (bias is a per-partition vector there), mm2's
      moving operand is the tanh output directly, and mm2 output lands d-major
      = the state layout. No PE transposes at all.
    - PSUM discipline: a start=True matmul logically zeroes its whole 2KB bank
      ("zero region"), so accumulation groups in one bank must be strictly
      sequential and bank reuse must wait for all readers. mm1 groups rotate
      over 4 banks (region hm//4) with the last 4 groups in 2 spare banks;
      mm2 over 2 banks. Violating this loses accumulated partial sums.
    - Bias enters via a coarse DVE add (pre-broadcast bf16 bias tile) per bank
      plus one coarse tanh per bank; the last 4 groups use fused tanh+bias so
      the mm1->mm2 transition tail is one short activation.
    - State accumulates in fp32 (DVE add from PSUM, deferred past mm2); a bf16
      shadow (zbf = bf16(zt + f), DVE, per d-tile) is the mm1 moving operand
      and unblocks the next step early (numerics: ~1.4e-3 final rel err).
    - Measured on 8 trn2 cores: ~233-278 us vs 851 us for variant a.

Variant "a": fp32 batch-stationary scheme (512-wide weight streams, PE
    transposes between the two matmuls). Exact but ~4x slower: fp32 matmuls
    cost 4 cyc/row, and fp32r is rejected by the ISA for the tile_position
    PSUM packing this layout needs (s3d3_mm_valid_dst_partition).

Sharding: pure data-parallel over batch (512 -> 8 x 64); weights replicated.
"""

import numpy as np

BS, D, HID = 512, 1024, 2048
NCORES = 8
B = BS // NCORES  # 64
NSTEP = 20
KD = D // 128  # 8 k-tiles for the D contraction
KH = HID // 128  # 16 k-tiles for the HID contraction
F32 = np.float32

VARIANT = "b"  # "b" (bf16 weights-stationary) or "a" (fp32 batch-stationary)
MM_DTYPE = "float32"  # variant a: "float32" or "float32r" (f32r fails ISA check)


# --------------------------------------------------------------------------
# Variant B: bf16 weights-stationary, no transposes
# --------------------------------------------------------------------------


def _build_program_b():
    import concourse.mybir as mybir
    from concourse import bacc
    from concourse.tile import TileContext

    nc = bacc.Bacc()
    f32 = mybir.dt.float32
    bf16 = mybir.dt.bfloat16
    TANH = mybir.ActivationFunctionType.Tanh
    COPY = mybir.ActivationFunctionType.Copy

    zt_in = nc.dram_tensor("zt_in", [128, KD * B], f32, kind="ExternalInput")
    zbf_in = nc.dram_tensor("zbf_in", [128, KD * B], bf16, kind="ExternalInput")
    w1_d = nc.dram_tensor("w1", [128, KD * HID], bf16, kind="ExternalInput")
    w2_d = nc.dram_tensor("w2", [128, KH * D], bf16, kind="ExternalInput")
    # biases packed per-partition: biases_d[p, i*KH + m] = bias_i[m*128 + p]
    biases_d = nc.dram_tensor("biases", [128, NSTEP * KH], f32, kind="ExternalInput")
    # bias values pre-broadcast over the batch for the coarse (per-bank) adds:
    # bias_tiled[p, i*768 + b*192 + r*64 + c] = bias_i[(4r+b)*128 + p], r<3
    biast_d = nc.dram_tensor("bias_tiled", [128, NSTEP * 768], bf16, kind="ExternalInput")
    zt_out = nc.dram_tensor("zt_out", [128, KD * B], f32, kind="ExternalOutput")

    with (
        TileContext(nc) as tc,
        tc.tile_pool(name="weights", bufs=1) as wpool,
        tc.tile_pool(name="state", bufs=1) as spool,
        tc.tile_pool(name="hbuf", bufs=2) as hpool,
        tc.tile_pool(name="psumh", bufs=1, space="PSUM") as ph_pool,
        tc.tile_pool(name="psumf", bufs=1, space="PSUM") as pf_pool,
    ):
        zt = spool.tile([128, KD * B], f32, tag="zt")
        zbf = spool.tile([128, KD * B], bf16, tag="zbf")
        bias_sb = wpool.tile([128, NSTEP * KH], f32, tag="bias")
        biast = wpool.tile([128, NSTEP * 768], bf16, tag="biast")

        # Weights live in per-group blocks (w1: one block per hm with all its
        # k-slices; w2: one block per dm with all its q-slices) so step 0's
        # hm-outer groups stream straight off the DMA arrival order.
        w1b = [
            wpool.tile([128, KD * 128], bf16, tag=f"w1_{hm}", name="w1b")
            for hm in range(KH)
        ]
        w2b = [
            wpool.tile([128, KH * 128], bf16, tag=f"w2_{dm}", name="w2b")
            for dm in range(KD)
        ]
        # DMA issues cost ~0.6us each on their issuing engine's sequencer;
        # spread them over three queues so step 0 isn't issue-serialized.
        nc.sync.dma_start(zbf[:], zbf_in[:])
        nc.scalar.dma_start(w1b[0][:], w1_d[:, 0 : KD * 128])
        nc.gpsimd.dma_start(bias_sb[:], biases_d[:])
        nc.gpsimd.dma_start(biast[:, 0 : 4 * 768], biast_d[:, 0 : 4 * 768])
        for hm in range(1, KH):
            eng = nc.sync if hm % 2 == 0 else nc.gpsimd
            eng.dma_start(w1b[hm][:], w1_d[:, hm * KD * 128 : (hm + 1) * KD * 128])
        nc.gpsimd.dma_start(zt[:], zt_in[:])
        for dm in range(KD):
            eng = nc.sync if dm % 2 == 0 else nc.gpsimd
            eng.dma_start(w2b[dm][:], w2_d[:, dm * KH * 128 : (dm + 1) * KH * 128])
        nc.sync.dma_start(biast[:, 4 * 768 :], biast_d[:, 4 * 768 :])

        for i in range(NSTEP):
            # ---- mm1: hT[hm] = bias_i[hm] + sum_k W1[k,hm]^T zbf[k] ----
            # PSUM rule: a start in a bank waits for stop of the bank's prior
            # group and all its readers (2KB zero region). Groups rotate over
            # 4 banks (hm%4, region hm//4 for hm<12); the last four groups
            # live in two spare banks so their starts don't WAR against the
            # coarse tanh reads of banks 0-3.
            ph = [
                ph_pool.tile(
                    [128, 3 * B], f32, tag=f"ph{b}", name="ph",
                    padded_shape=[128, 8 * B],
                )
                for b in range(4)
            ]
            # phx banks host the last four mm1 groups (regions 0-1) and are
            # reused for half the mm2 groups (regions 2-3) once the fused
            # tanhs have consumed them.
            phx = [
                ph_pool.tile(
                    [128, 4 * B], f32, tag=f"phx{b}", name="phx",
                    padded_shape=[128, 8 * B],
                )
                for b in range(2)
            ]
            h_bf = hpool.tile([128, KH * B], bf16, tag="hbf")

            def ph_ap(hm):
                if hm >= 12:
                    e = hm - 12
                    return phx[e % 2][:, (e // 2) * B : (e // 2) * B + B]
                return ph[hm // 3][:, (hm % 3) * B : (hm % 3) * B + B]
            for hm in range(KH):
                for k in range(KD):
                    nc.tensor.matmul(
                        ph_ap(hm),
                        w1b[hm][:, k * 128 : k * 128 + 128],
                        zbf[:, k * B : k * B + B],
                        start=(k == 0),
                        stop=(k == KD - 1),
                    )
                if hm >= 12:
                    # last four groups: fused tanh(x + bias) per region so the
                    # tail after the final group is one short activation
                    nc.scalar.activation(
                        h_bf[:, hm * B : hm * B + B],
                        ph_ap(hm),
                        TANH,
                        bias=bias_sb[:, i * KH + hm : i * KH + hm + 1],
                    )
                elif hm % 3 == 2:
                    # bank b=hm//3 holds the three consecutive groups 3b..3b+2
                    # and just completed: coarse DVE bias-add then one coarse
                    # tanh over [128, 192]. Early banks finish 3 groups in, so
                    # h production trails mm1 by only ~3 groups.
                    b = hm // 3
                    nc.vector.tensor_add(
                        ph[b][:, 0 : 3 * B],
                        ph[b][:, 0 : 3 * B],
                        biast[:, i * 768 + b * 192 : i * 768 + b * 192 + 192],
                    )
                    nc.scalar.activation(
                        h_bf[:, 3 * b * B : (3 * b + 3) * B],
                        ph[b][:, 0 : 3 * B],
                        TANH,
                    )

            # ---- mm2: f[dm] = sum_q W2'[q,dm]^T h[q]; d-major PSUM ----
            # Spread the 8 dm groups over FOUR tiles (2 pf banks + the 2 phx
            # banks): a start=True matmul conflicts with its whole tile, so a
            # tile's next start waits on the previous group's adds; 4 tiles
            # give that chain a 3-group window instead of 1.
            pf = [
                pf_pool.tile(
                    [128, 2 * B], f32, tag=f"pf{b}", name="pf",
                    padded_shape=[128, 8 * B],
                )
                for b in range(2)
            ]

            def pf_ap(dm):
                t = dm % 4
                r = dm // 4
                if t < 2:
                    return pf[t][:, r * B : r * B + B]
                return phx[t - 2][:, (2 + r) * B : (2 + r) * B + B]

            # plain ascending q-order already matches h availability (coarse
            # tanh of bank b yields chunks 3b..3b+2 in order, then fused 12-15)
            for dm in range(KD):
                for qi, q in enumerate(range(KH)):
                    nc.tensor.matmul(
                        pf_ap(dm),
                        w2b[dm][:, q * 128 : q * 128 + 128],
                        h_bf[:, q * B : q * B + B],
                        start=(qi == 0),
                        stop=(qi == KH - 1),
                    )
                # bf16 shadow per-dm (critical path: feeds next step's mm1;
                # dead on the final step)
                if i < NSTEP - 1:
                    nc.vector.tensor_add(
                        zbf[:, dm * B : dm * B + B],
                        zt[:, dm * B : dm * B + B],
                        pf_ap(dm),
                    )
                else:
                    nc.vector.tensor_add(
                        zt[:, dm * B : dm * B + B],
                        zt[:, dm * B : dm * B + B],
                        pf_ap(dm),
                    )
                    if dm == 3:
                        nc.sync.dma_start(
                            zt_out[:, 0 : 4 * B], zt[:, 0 : 4 * B]
                        )
            # fp32 state updates deferred past mm2: they read the pf banks,
            # and the pf banks' next starts are a full step away, while the
            # mm2 phase keeps DVE traffic off the banks the PE accumulates.
            # (On the final step the zt-adds ran inline above instead, so the
            # kernel tail is just the output DMA.)
            if i < NSTEP - 1:
                for dm in range(KD):
                    nc.vector.tensor_add(
                        zt[:, dm * B : dm * B + B],
                        zt[:, dm * B : dm * B + B],
                        pf_ap(dm),
                    )

        nc.sync.dma_start(zt_out[:, 4 * B :], zt[:, 4 * B :])

    nc.compile()
    return nc


# --------------------------------------------------------------------------
# Variant A: fp32 batch-stationary (original baseline)
# --------------------------------------------------------------------------


def _build_program_a(mm_dtype=MM_DTYPE, repeat=1):
    import concourse.mybir as mybir
    from concourse import bacc
    from concourse.tile import TileContext

    nc = bacc.Bacc()
    f32 = mybir.dt.float32
    mmdt = getattr(mybir.dt, mm_dtype)
    TANH = mybir.ActivationFunctionType.Tanh

    zt_in = nc.dram_tensor("zt_in", [128, KD * B], mmdt, kind="ExternalInput")
    w1_d = nc.dram_tensor("w1", [128, KD * HID], mmdt, kind="ExternalInput")
    w2_d = nc.dram_tensor("w2", [128, KH * D], mmdt, kind="ExternalInput")
    biases_d = nc.dram_tensor("biases", [NSTEP, HID], mmdt, kind="ExternalInput")
    ident_d = nc.dram_tensor("ident", [128, 128], mmdt, kind="ExternalInput")
    ones_d = nc.dram_tensor("ones", [1, B], mmdt, kind="ExternalInput")
    zt_out = nc.dram_tensor("zt_out", [128, KD * B], mmdt, kind="ExternalOutput")

    with (
        TileContext(nc) as tc,
        tc.tile_pool(name="const", bufs=1) as cpool,
        tc.tile_pool(name="weights", bufs=1) as wpool,
        tc.tile_pool(name="state", bufs=1) as spool,
        tc.tile_pool(name="work", bufs=2) as hpool,
        tc.tile_pool(name="bias", bufs=2) as bpool,
        tc.tile_pool(name="psumh", bufs=2, space="PSUM") as ph_pool,
        tc.tile_pool(name="psumt", bufs=2, space="PSUM") as pt_pool,
        tc.tile_pool(name="psumf", bufs=2, space="PSUM") as pf_pool,
    ):
        ident_sb = cpool.tile([128, 128], mmdt, tag="ident")
        nc.sync.dma_start(ident_sb[:], ident_d[:])
        ones_sb = cpool.tile([1, B], mmdt, tag="ones")
        nc.sync.dma_start(ones_sb[:], ones_d[:])

        zt = spool.tile([128, KD * B], mmdt, tag="zt")
        nc.sync.dma_start(zt[:], zt_in[:])
        hT = spool.tile([128, KH * B], mmdt, tag="hT")

        w1t = []
        for k in range(KD):
            w = wpool.tile([128, HID], mmdt, tag=f"w1_{k}")
            nc.sync.dma_start(w[:], w1_d[:, k * HID : (k + 1) * HID])
            w1t.append(w)
        w2t = []
        for k in range(KH):
            w = wpool.tile([128, D], mmdt, tag=f"w2_{k}")
            nc.sync.dma_start(w[:], w2_d[:, k * D : (k + 1) * D])
            w2t.append(w)

        def scan_body(_iv=None):
            for i in range(NSTEP):
                bias_sb = bpool.tile([1, HID], mmdt, tag="bias")
                nc.sync.dma_start(bias_sb[:], biases_d[i : i + 1, :])

                phs = []
                for g in range(2):
                    ph = ph_pool.tile([128, 512], f32, tag="ph")
                    phs.append(ph)
                    for half in range(2):
                        c = 2 * g + half
                        nc.tensor.matmul(
                            ph[64 * half : 64 * half + 64, :],
                            ones_sb[:1, :],
                            bias_sb[:1, 512 * c : 512 * c + 512],
                            start=True,
                            stop=False,
                            tile_position=(0, 64 * half),
                        )
                    for k in range(KD):
                        for half in range(2):
                            c = 2 * g + half
                            nc.tensor.matmul(
                                ph[64 * half : 64 * half + 64, :],
                                zt[:, B * k : B * k + B],
                                w1t[k][:, 512 * c : 512 * c + 512],
                                start=False,
                                stop=(k == KD - 1),
                                tile_position=(0, 64 * half),
                            )

                for g in range(2):
                    h_bm = hpool.tile([128, 512], mmdt, tag="h_bm")
                    nc.scalar.activation(h_bm[:], phs[g][:], TANH)
                    pt = pt_pool.tile([128, 512], mmdt, tag="pt")
                    for u in range(4):
                        nc.tensor.matmul(
                            pt[:, 128 * u : 128 * u + 128],
                            h_bm[:, 128 * u : 128 * u + 128],
                            ident_sb[:],
                            is_transpose=True,
                            start=True,
                            stop=True,
                        )
                    nc.vector.tensor_copy(
                        hT[:, 512 * g : 512 * g + 512].rearrange(
                            "p (h u c) -> p h u c", h=2, u=4
                        ),
                        pt[:].rearrange("p (u h c) -> p h u c", u=4, h=2),
                    )

                pf = pf_pool.tile([128, 512], f32, tag="pf")
                for k in range(KH):
                    for half in range(2):
                        nc.tensor.matmul(
                            pf[64 * half : 64 * half + 64, :],
                            hT[:, B * k : B * k + B],
                            w2t[k][:, 512 * half : 512 * half + 512],
                            start=(k == 0),
                            stop=(k == KH - 1),
                            tile_position=(0, 64 * half),
                        )

                f_bm = hpool.tile([128, 512], mmdt, tag="f_bm")
                nc.vector.tensor_copy(f_bm[:], pf[:])
                pt2 = pt_pool.tile([128, 512], mmdt, tag="pt")
                for u in range(4):
                    nc.tensor.matmul(
                        pt2[:, 128 * u : 128 * u + 128],
                        f_bm[:, 128 * u : 128 * u + 128],
                        ident_sb[:],
                        is_transpose=True,
                        start=True,
                        stop=True,
                    )
                zt_v = zt[:].rearrange("p (h u c) -> p h u c", h=2, u=4)
                nc.vector.tensor_add(
                    zt_v, zt_v, pt2[:].rearrange("p (u h c) -> p h u c", u=4, h=2)
                )

        if repeat == 1:
            scan_body()
        else:
            with tc.For_i(0, repeat, 1) as _i:
                scan_body(_i)

        nc.sync.dma_start(zt_out[:, 4 * B :], zt[:, 4 * B :])

    nc.compile()
    return nc


# --------------------------------------------------------------------------
# Host-side packing
# --------------------------------------------------------------------------


def _pack_zT(shard):  # [B, D] -> [128, KD*B]
    return np.ascontiguousarray(
        shard.T.reshape(KD, 128, B).transpose(1, 0, 2).reshape(128, KD * B)
    )


def _unpack_zT(zt):  # [128, KD*B] -> [B, D]
    return zt.reshape(128, KD, B).transpose(1, 0, 2).reshape(D, B).T


def _host_common(z0, t, W1, b1, wt, W2, b2):
    t = np.asarray(t, F32)
    t0s, t1s = t[:-1], t[1:]
    h_seg = (t1s - t0s) / 2.0  # N_STEPS_PER_SEG = 2
    step_ts = (
        t0s[:, None] + h_seg[:, None] * np.arange(2, dtype=F32)[None, :]
    ).reshape(-1)
    step_hs = np.repeat(h_seg, 2)
    assert np.allclose(step_hs, step_hs[0]), "non-uniform Euler steps unsupported"
    scale = F32(step_hs[0])

    c = (scale * np.asarray(b2, F32)).astype(F32)  # [D]
    cW1 = (c.astype(np.float64) @ np.asarray(W1, np.float64)).astype(F32)  # [HID]
    biases = np.stack(
        [
            (np.asarray(b1, F32) + step_ts[i] * np.asarray(wt, F32) + i * cW1).astype(
                F32
            )
            for i in range(NSTEP)
        ]
    )  # [NSTEP, HID]
    return biases, scale, c


def _make_in_maps_b(z0, t, W1, b1, wt, W2, b2):
    import ml_dtypes

    bf16 = ml_dtypes.bfloat16
    z0 = np.asarray(z0, F32)
    biases, scale, c = _host_common(z0, t, W1, b1, wt, W2, b2)

    bias_cols = np.ascontiguousarray(
        biases.reshape(NSTEP, KH, 128).transpose(2, 0, 1).reshape(128, NSTEP * KH)
    )
    # bias_tiled[p, i*768 + b*192 + r*64 + c] = biases[i, (3b+r)*128 + p], r<3
    A = biases.reshape(NSTEP, KH, 128)[:, :12, :].reshape(NSTEP, 4, 3, 128)
    bias_tiled = np.ascontiguousarray(
        np.broadcast_to(
            A.transpose(3, 0, 1, 2)[:, :, :, :, None], (128, NSTEP, 4, 3, B)
        ).reshape(128, NSTEP * 768)
    ).astype(bf16)
    # w1p[p, hm*KD*128 + k*128 + c] = W1[k*128+p, hm*128+c]
    w1p = np.ascontiguousarray(
        np.asarray(W1, F32)
        .reshape(KD, 128, KH, 128)
        .transpose(1, 2, 0, 3)
        .reshape(128, KD * HID)
    ).astype(bf16)
    # w2p[p, dm*KH*128 + q*128 + c] = W2'[q*128+p, dm*128+c]
    w2p = np.ascontiguousarray(
        (scale * np.asarray(W2, F32))
        .astype(F32)
        .reshape(KH, 128, KD, 128)
        .transpose(1, 2, 0, 3)
        .reshape(128, KH * D)
    ).astype(bf16)

    in_maps = []
    for core in range(NCORES):
        shard = z0[core * B : (core + 1) * B]
        ztp = _pack_zT(shard)
        in_maps.append(
            {
                "zt_in": ztp,
                "zbf_in": ztp.astype(bf16),
                "w1": w1p,
                "w2": w2p,
                "biases": bias_cols,
                "bias_tiled": bias_tiled,
            }
        )
    return in_maps, c


def _make_in_maps_a(z0, t, W1, b1, wt, W2, b2):
    z0 = np.asarray(z0, F32)
    biases, scale, c = _host_common(z0, t, W1, b1, wt, W2, b2)
    w1p = np.ascontiguousarray(
        np.asarray(W1, F32)
        .reshape(KD, 128, HID)
        .transpose(1, 0, 2)
        .reshape(128, KD * HID)
    )
    w2p = np.ascontiguousarray(
        (scale * np.asarray(W2, F32))
        .astype(F32)
        .reshape(KH, 128, D)
        .transpose(1, 0, 2)
        .reshape(128, KH * D)
    )
    ident = np.eye(128, dtype=F32)
    ones = np.ones((1, B), F32)
    in_maps = []
    for core in range(NCORES):
        shard = z0[core * B : (core + 1) * B]
        in_maps.append(
            {
                "zt_in": _pack_zT(shard),
                "w1": w1p,
                "w2": w2p,
                "biases": biases,
                "ident": ident,
                "ones": ones,
            }
        )
    return in_maps, c


def run(z0, t, W1, b1, wt, W2, b2, trace=False, mm_dtype=MM_DTYPE, variant=VARIANT):
    from concourse.bass_utils import run_bass_kernel_spmd

    if variant == "b":
        in_maps, c = _make_in_maps_b(z0, t, W1, b1, wt, W2, b2)
        nc = _build_program_b()
    else:
        in_maps, c = _make_in_maps_a(z0, t, W1, b1, wt, W2, b2)
        nc = _build_program_a(mm_dtype=mm_dtype)
    res = run_bass_kernel_spmd(nc, in_maps, core_ids=list(range(NCORES)), trace=trace)

    outs = []
    for core in range(NCORES):
        z_shard = _unpack_zT(np.asarray(res.results[core]["zt_out"], F32))
        outs.append(z_shard)
    out = np.concatenate(outs, axis=0).astype(F32)
    out = out + (NSTEP * c)[None, :].astype(F32)
    return out.astype(F32), res


def kernel(z0, t, W1, b1, wt, W2, b2):
    out, _ = run(z0, t, W1, b1, wt, W2, b2, trace=False)
    return out


# revision 33
# speedup vs baseline: 1.0369x; 1.0218x over previous
"""Trainium2 Bass kernel for the NeuralODE problem.

Math (matching reference.py):
    20 Euler steps (10 segments x 2 steps, uniform dt => step size hi = 0.05):
        z_{i+1} = z_i + hi * ( tanh(z_i @ W1 + b1 + t_i*wt) @ W2 + b2 )

Shared device-side reformulation (per core, batch shard B=64):
    - Fold hi into W2:  W2' = hi * W2, c = hi * b2.
    - Keep the "state without accumulated c":  z'_i = z_i - i*c, so
        z'_{i+1} = z'_i + tanh(z'_i @ W1 + bias_i) @ W2'
      with bias_i = b1 + t_i*wt + i*(c @ W1)   (precomputed on host).
      Final output: z_20 = z'_20 + 20*c       (added on host).
    - State kept transposed (d-major) as zT[p, 64k+b] = z'[b, 128k+p].

Variant "b" (default): bf16 weights-STATIONARY scheme, ~3x the fp32 variant.
    - Both matmuls keep a [128,128] weight block as the stationary operand and
      stream the 64-wide batch as the moving operand (bf16 => 1 cyc/row, full
      128x128 PE utilization, LDWEIGHTS hides under the previous matmul).
    - mm1 output lands hid-major (bias is a per-partition vector there), mm2's
      moving operand is the tanh output directly, and mm2 output lands d-major
      = the state layout. No PE transposes at all.
    - PSUM discipline: a start=True matmul logically zeroes its whole 2KB bank
      ("zero region"), so accumulation groups in one bank must be strictly
      sequential and bank reuse must wait for all readers. mm1 groups rotate
      over 4 banks (region hm//4) with the last 4 groups in 2 spare banks;
      mm2 over 2 banks. Violating this loses accumulated partial sums.
    - Bias enters via a coarse DVE add (pre-broadcast bf16 bias tile) per bank
      plus one coarse tanh per bank; the last 4 groups use fused tanh+bias so
      the mm1->mm2 transition tail is one short activation.
    - State accumulates in fp32 (DVE add from PSUM, deferred past mm2); a bf16
      shadow (zbf = bf16(zt + f), DVE, per d-tile) is the mm1 moving operand
      and unblocks the next step early (numerics: ~1.4e-3 final rel err).
    - Measured on 8 trn2 cores: ~233-278 us vs 851 us for variant a.

Variant "a": fp32 batch-stationary scheme (512-wide weight streams, PE
    transposes between the two matmuls). Exact but ~4x slower: fp32 matmuls
    cost 4 cyc/row, and fp32r is rejected by the ISA for the tile_position
    PSUM packing this layout needs (s3d3_mm_valid_dst_partition).

Sharding: pure data-parallel over batch (512 -> 8 x 64); weights replicated.
"""

import numpy as np

BS, D, HID = 512, 1024, 2048
NCORES = 8
B = BS // NCORES  # 64
NSTEP = 20
KD = D // 128  # 8 k-tiles for the D contraction
KH = HID // 128  # 16 k-tiles for the HID contraction
F32 = np.float32

VARIANT = "b"  # "b" (bf16 weights-stationary) or "a" (fp32 batch-stationary)
MM_DTYPE = "float32"  # variant a: "float32" or "float32r" (f32r fails ISA check)


# --------------------------------------------------------------------------
# Variant B: bf16 weights-stationary, no transposes
# --------------------------------------------------------------------------


def _build_program_b():
    import concourse.mybir as mybir
    from concourse import bacc
    from concourse.tile import TileContext

    nc = bacc.Bacc()
    f32 = mybir.dt.float32
    bf16 = mybir.dt.bfloat16
    TANH = mybir.ActivationFunctionType.Tanh
    COPY = mybir.ActivationFunctionType.Copy

    zt_in = nc.dram_tensor("zt_in", [128, KD * B], f32, kind="ExternalInput")
    zbf_in = nc.dram_tensor("zbf_in", [128, KD * B], bf16, kind="ExternalInput")
    w1_d = nc.dram_tensor("w1", [128, KD * HID], bf16, kind="ExternalInput")
    w2_d = nc.dram_tensor("w2", [128, KH * D], bf16, kind="ExternalInput")
    # biases packed per-partition: biases_d[p, i*KH + m] = bias_i[m*128 + p]
    biases_d = nc.dram_tensor("biases", [128, NSTEP * KH], f32, kind="ExternalInput")
    # bias values pre-broadcast over the batch for the coarse (per-bank) adds:
    # bias_tiled[p, i*768 + b*192 + r*64 + c] = bias_i[(4r+b)*128 + p], r<3
    biast_d = nc.dram_tensor("bias_tiled", [128, NSTEP * 768], bf16, kind="ExternalInput")
    zt_out = nc.dram_tensor("zt_out", [128, KD * B], f32, kind="ExternalOutput")

    with (
        TileContext(nc) as tc,
        tc.tile_pool(name="weights", bufs=1) as wpool,
        tc.tile_pool(name="state", bufs=1) as spool,
        tc.tile_pool(name="hbuf", bufs=2) as hpool,
        tc.tile_pool(name="psumh", bufs=1, space="PSUM") as ph_pool,
        tc.tile_pool(name="psumf", bufs=1, space="PSUM") as pf_pool,
    ):
        zt = spool.tile([128, KD * B], f32, tag="zt")
        zbf = spool.tile([128, KD * B], bf16, tag="zbf")
        bias_sb = wpool.tile([128, NSTEP * KH], f32, tag="bias")
        biast = wpool.tile([128, NSTEP * 768], bf16, tag="biast")

        # Weights live in per-group blocks (w1: one block per hm with all its
        # k-slices; w2: one block per dm with all its q-slices) so step 0's
        # hm-outer groups stream straight off the DMA arrival order.
        w1b = [
            wpool.tile([128, KD * 128], bf16, tag=f"w1_{hm}", name="w1b")
            for hm in range(KH)
        ]
        w2b = [
            wpool.tile([128, KH * 128], bf16, tag=f"w2_{dm}", name="w2b")
            for dm in range(KD)
        ]
        nc.sync.dma_start(zbf[:], zbf_in[:])
        nc.sync.dma_start(bias_sb[:], biases_d[:])
        nc.sync.dma_start(w1b[0][:], w1_d[:, 0 : KD * 128])
        nc.sync.dma_start(biast[:, 0 : 4 * 768], biast_d[:, 0 : 4 * 768])
        for hm in range(1, KH):
            nc.sync.dma_start(w1b[hm][:], w1_d[:, hm * KD * 128 : (hm + 1) * KD * 128])
        nc.sync.dma_start(zt[:], zt_in[:])
        for dm in range(KD):
            nc.sync.dma_start(w2b[dm][:], w2_d[:, dm * KH * 128 : (dm + 1) * KH * 128])
        nc.sync.dma_start(biast[:, 4 * 768 :], biast_d[:, 4 * 768 :])

        for i in range(NSTEP):
            # ---- mm1: hT[hm] = bias_i[hm] + sum_k W1[k,hm]^T zbf[k] ----
            # PSUM rule: a start in a bank waits for stop of the bank's prior
            # group and all its readers (2KB zero region). Groups rotate over
            # 4 banks (hm%4, region hm//4 for hm<12); the last four groups
            # live in two spare banks so their starts don't WAR against the
            # coarse tanh reads of banks 0-3.
            ph = [
                ph_pool.tile(
                    [128, 3 * B], f32, tag=f"ph{b}", name="ph",
                    padded_shape=[128, 8 * B],
                )
                for b in range(4)
            ]
            # phx banks host the last four mm1 groups (regions 0-1) and are
            # reused for half the mm2 groups (regions 2-3) once the fused
            # tanhs have consumed them.
            phx = [
                ph_pool.tile(
                    [128, 4 * B], f32, tag=f"phx{b}", name="phx",
                    padded_shape=[128, 8 * B],
                )
                for b in range(2)
            ]
            h_bf = hpool.tile([128, KH * B], bf16, tag="hbf")

            def ph_ap(hm):
                if hm >= 12:
                    e = hm - 12
                    return phx[e % 2][:, (e // 2) * B : (e // 2) * B + B]
                return ph[hm // 3][:, (hm % 3) * B : (hm % 3) * B + B]
            for hm in range(KH):
                for k in range(KD):
                    nc.tensor.matmul(
                        ph_ap(hm),
                        w1b[hm][:, k * 128 : k * 128 + 128],
                        zbf[:, k * B : k * B + B],
                        start=(k == 0),
                        stop=(k == KD - 1),
                    )
                if hm >= 12:
                    # last four groups: fused tanh(x + bias) per region so the
                    # tail after the final group is one short activation
                    nc.scalar.activation(
                        h_bf[:, hm * B : hm * B + B],
                        ph_ap(hm),
                        TANH,
                        bias=bias_sb[:, i * KH + hm : i * KH + hm + 1],
                    )
                elif hm % 3 == 2:
                    # bank b=hm//3 holds the three consecutive groups 3b..3b+2
                    # and just completed: coarse DVE bias-add then one coarse
                    # tanh over [128, 192]. Early banks finish 3 groups in, so
                    # h production trails mm1 by only ~3 groups.
                    b = hm // 3
                    nc.vector.tensor_add(
                        ph[b][:, 0 : 3 * B],
                        ph[b][:, 0 : 3 * B],
                        biast[:, i * 768 + b * 192 : i * 768 + b * 192 + 192],
                    )
                    nc.scalar.activation(
                        h_bf[:, 3 * b * B : (3 * b + 3) * B],
                        ph[b][:, 0 : 3 * B],
                        TANH,
                    )

            # ---- mm2: f[dm] = sum_q W2'[q,dm]^T h[q]; d-major PSUM ----
            # Spread the 8 dm groups over FOUR tiles (2 pf banks + the 2 phx
            # banks): a start=True matmul conflicts with its whole tile, so a
            # tile's next start waits on the previous group's adds; 4 tiles
            # give that chain a 3-group window instead of 1.
            pf = [
                pf_pool.tile(
                    [128, 2 * B], f32, tag=f"pf{b}", name="pf",
                    padded_shape=[128, 8 * B],
                )
                for b in range(2)
            ]

            def pf_ap(dm):
                t = dm % 4
                r = dm // 4
                if t < 2:
                    return pf[t][:, r * B : r * B + B]
                return phx[t - 2][:, (2 + r) * B : (2 + r) * B + B]

            # plain ascending q-order already matches h availability (coarse
            # tanh of bank b yields chunks 3b..3b+2 in order, then fused 12-15)
            for dm in range(KD):
                for qi, q in enumerate(range(KH)):
                    nc.tensor.matmul(
                        pf_ap(dm),
                        w2b[dm][:, q * 128 : q * 128 + 128],
                        h_bf[:, q * B : q * B + B],
                        start=(qi == 0),
                        stop=(qi == KH - 1),
                    )
                # bf16 shadow per-dm (critical path: feeds next step's mm1;
                # dead on the final step)
                if i < NSTEP - 1:
                    nc.vector.tensor_add(
                        zbf[:, dm * B : dm * B + B],
                        zt[:, dm * B : dm * B + B],
                        pf_ap(dm),
                    )
                else:
                    nc.vector.tensor_add(
                        zt[:, dm * B : dm * B + B],
                        zt[:, dm * B : dm * B + B],
                        pf_ap(dm),
                    )
            # fp32 state updates deferred past mm2: they read the pf banks,
            # and the pf banks' next starts are a full step away, while the
            # mm2 phase keeps DVE traffic off the banks the PE accumulates.
            # (On the final step the zt-adds ran inline above instead, so the
            # kernel tail is just the output DMA.)
            if i < NSTEP - 1:
                for dm in range(KD):
                    nc.vector.tensor_add(
                        zt[:, dm * B : dm * B + B],
                        zt[:, dm * B : dm * B + B],
                        pf_ap(dm),
                    )

        nc.sync.dma_start(zt_out[:], zt[:])

    nc.compile()
    return nc


# --------------------------------------------------------------------------
# Variant A: fp32 batch-stationary (original baseline)
# --------------------------------------------------------------------------


def _build_program_a(mm_dtype=MM_DTYPE, repeat=1):
    import concourse.mybir as mybir
    from concourse import bacc
    from concourse.tile import TileContext

    nc = bacc.Bacc()
    f32 = mybir.dt.float32
    mmdt = getattr(mybir.dt, mm_dtype)
    TANH = mybir.ActivationFunctionType.Tanh

    zt_in = nc.dram_tensor("zt_in", [128, KD * B], mmdt, kind="ExternalInput")
    w1_d = nc.dram_tensor("w1", [128, KD * HID], mmdt, kind="ExternalInput")
    w2_d = nc.dram_tensor("w2", [128, KH * D], mmdt, kind="ExternalInput")
    biases_d = nc.dram_tensor("biases", [NSTEP, HID], mmdt, kind="ExternalInput")
    ident_d = nc.dram_tensor("ident", [128, 128], mmdt, kind="ExternalInput")
    ones_d = nc.dram_tensor("ones", [1, B], mmdt, kind="ExternalInput")
    zt_out = nc.dram_tensor("zt_out", [128, KD * B], mmdt, kind="ExternalOutput")

    with (
        TileContext(nc) as tc,
        tc.tile_pool(name="const", bufs=1) as cpool,
        tc.tile_pool(name="weights", bufs=1) as wpool,
        tc.tile_pool(name="state", bufs=1) as spool,
        tc.tile_pool(name="work", bufs=2) as hpool,
        tc.tile_pool(name="bias", bufs=2) as bpool,
        tc.tile_pool(name="psumh", bufs=2, space="PSUM") as ph_pool,
        tc.tile_pool(name="psumt", bufs=2, space="PSUM") as pt_pool,
        tc.tile_pool(name="psumf", bufs=2, space="PSUM") as pf_pool,
    ):
        ident_sb = cpool.tile([128, 128], mmdt, tag="ident")
        nc.sync.dma_start(ident_sb[:], ident_d[:])
        ones_sb = cpool.tile([1, B], mmdt, tag="ones")
        nc.sync.dma_start(ones_sb[:], ones_d[:])

        zt = spool.tile([128, KD * B], mmdt, tag="zt")
        nc.sync.dma_start(zt[:], zt_in[:])
        hT = spool.tile([128, KH * B], mmdt, tag="hT")

        w1t = []
        for k in range(KD):
            w = wpool.tile([128, HID], mmdt, tag=f"w1_{k}")
            nc.sync.dma_start(w[:], w1_d[:, k * HID : (k + 1) * HID])
            w1t.append(w)
        w2t = []
        for k in range(KH):
            w = wpool.tile([128, D], mmdt, tag=f"w2_{k}")
            nc.sync.dma_start(w[:], w2_d[:, k * D : (k + 1) * D])
            w2t.append(w)

        def scan_body(_iv=None):
            for i in range(NSTEP):
                bias_sb = bpool.tile([1, HID], mmdt, tag="bias")
                nc.sync.dma_start(bias_sb[:], biases_d[i : i + 1, :])

                phs = []
                for g in range(2):
                    ph = ph_pool.tile([128, 512], f32, tag="ph")
                    phs.append(ph)
                    for half in range(2):
                        c = 2 * g + half
                        nc.tensor.matmul(
                            ph[64 * half : 64 * half + 64, :],
                            ones_sb[:1, :],
                            bias_sb[:1, 512 * c : 512 * c + 512],
                            start=True,
                            stop=False,
                            tile_position=(0, 64 * half),
                        )
                    for k in range(KD):
                        for half in range(2):
                            c = 2 * g + half
                            nc.tensor.matmul(
                                ph[64 * half : 64 * half + 64, :],
                                zt[:, B * k : B * k + B],
                                w1t[k][:, 512 * c : 512 * c + 512],
                                start=False,
                                stop=(k == KD - 1),
                                tile_position=(0, 64 * half),
                            )

                for g in range(2):
                    h_bm = hpool.tile([128, 512], mmdt, tag="h_bm")
                    nc.scalar.activation(h_bm[:], phs[g][:], TANH)
                    pt = pt_pool.tile([128, 512], mmdt, tag="pt")
                    for u in range(4):
                        nc.tensor.matmul(
                            pt[:, 128 * u : 128 * u + 128],
                            h_bm[:, 128 * u : 128 * u + 128],
                            ident_sb[:],
                            is_transpose=True,
                            start=True,
                            stop=True,
                        )
                    nc.vector.tensor_copy(
                        hT[:, 512 * g : 512 * g + 512].rearrange(
                            "p (h u c) -> p h u c", h=2, u=4
                        ),
                        pt[:].rearrange("p (u h c) -> p h u c", u=4, h=2),
                    )

                pf = pf_pool.tile([128, 512], f32, tag="pf")
                for k in range(KH):
                    for half in range(2):
                        nc.tensor.matmul(
                            pf[64 * half : 64 * half + 64, :],
                            hT[:, B * k : B * k + B],
                            w2t[k][:, 512 * half : 512 * half + 512],
                            start=(k == 0),
                            stop=(k == KH - 1),
                            tile_position=(0, 64 * half),
                        )

                f_bm = hpool.tile([128, 512], mmdt, tag="f_bm")
                nc.vector.tensor_copy(f_bm[:], pf[:])
                pt2 = pt_pool.tile([128, 512], mmdt, tag="pt")
                for u in range(4):
                    nc.tensor.matmul(
                        pt2[:, 128 * u : 128 * u + 128],
                        f_bm[:, 128 * u : 128 * u + 128],
                        ident_sb[:],
                        is_transpose=True,
                        start=True,
                        stop=True,
                    )
                zt_v = zt[:].rearrange("p (h u c) -> p h u c", h=2, u=4)
                nc.vector.tensor_add(
                    zt_v, zt_v, pt2[:].rearrange("p (u h c) -> p h u c", u=4, h=2)
                )

        if repeat == 1:
            scan_body()
        else:
            with tc.For_i(0, repeat, 1) as _i:
                scan_body(_i)

        nc.sync.dma_start(zt_out[:], zt[:])

    nc.compile()
    return nc


# --------------------------------------------------------------------------
# Host-side packing
# --------------------------------------------------------------------------


def _pack_zT(shard):  # [B, D] -> [128, KD*B]
    return np.ascontiguousarray(
        shard.T.reshape(KD, 128, B).transpose(1, 0, 2).reshape(128, KD * B)
    )


def _unpack_zT(zt):  # [128, KD*B] -> [B, D]
    return zt.reshape(128, KD, B).transpose(1, 0, 2).reshape(D, B).T


def _host_common(z0, t, W1, b1, wt, W2, b2):
    t = np.asarray(t, F32)
    t0s, t1s = t[:-1], t[1:]
    h_seg = (t1s - t0s) / 2.0  # N_STEPS_PER_SEG = 2
    step_ts = (
        t0s[:, None] + h_seg[:, None] * np.arange(2, dtype=F32)[None, :]
    ).reshape(-1)
    step_hs = np.repeat(h_seg, 2)
    assert np.allclose(step_hs, step_hs[0]), "non-uniform Euler steps unsupported"
    scale = F32(step_hs[0])

    c = (scale * np.asarray(b2, F32)).astype(F32)  # [D]
    cW1 = (c.astype(np.float64) @ np.asarray(W1, np.float64)).astype(F32)  # [HID]
    biases = np.stack(
        [
            (np.asarray(b1, F32) + step_ts[i] * np.asarray(wt, F32) + i * cW1).astype(
                F32
            )
            for i in range(NSTEP)
        ]
    )  # [NSTEP, HID]
    return biases, scale, c


def _make_in_maps_b(z0, t, W1, b1, wt, W2, b2):
    import ml_dtypes

    bf16 = ml_dtypes.bfloat16
    z0 = np.asarray(z0, F32)
    biases, scale, c = _host_common(z0, t, W1, b1, wt, W2, b2)

    bias_cols = np.ascontiguousarray(
        biases.reshape(NSTEP, KH, 128).transpose(2, 0, 1).reshape(128, NSTEP * KH)
    )
    # bias_tiled[p, i*768 + b*192 + r*64 + c] = biases[i, (3b+r)*128 + p], r<3
    A = biases.reshape(NSTEP, KH, 128)[:, :12, :].reshape(NSTEP, 4, 3, 128)
    bias_tiled = np.ascontiguousarray(
        np.broadcast_to(
            A.transpose(3, 0, 1, 2)[:, :, :, :, None], (128, NSTEP, 4, 3, B)
        ).reshape(128, NSTEP * 768)
    ).astype(bf16)
    # w1p[p, hm*KD*128 + k*128 + c] = W1[k*128+p, hm*128+c]
    w1p = np.ascontiguousarray(
        np.asarray(W1, F32)
        .reshape(KD, 128, KH, 128)
        .transpose(1, 2, 0, 3)
        .reshape(128, KD * HID)
    ).astype(bf16)
    # w2p[p, dm*KH*128 + q*128 + c] = W2'[q*128+p, dm*128+c]
    w2p = np.ascontiguousarray(
        (scale * np.asarray(W2, F32))
        .astype(F32)
        .reshape(KH, 128, KD, 128)
        .transpose(1, 2, 0, 3)
        .reshape(128, KH * D)
    ).astype(bf16)

    in_maps = []
    for core in range(NCORES):
        shard = z0[core * B : (core + 1) * B]
        ztp = _pack_zT(shard)
        in_maps.append(
            {
                "zt_in": ztp,
                "zbf_in": ztp.astype(bf16),
                "w1": w1p,
                "w2": w2p,
                "biases": bias_cols,
                "bias_tiled": bias_tiled,
            }
        )
    return in_maps, c


def _make_in_maps_a(z0, t, W1, b1, wt, W2, b2):
    z0 = np.asarray(z0, F32)
    biases, scale, c = _host_common(z0, t, W1, b1, wt, W2, b2)
    w1p = np.ascontiguousarray(
        np.asarray(W1, F32)
        .reshape(KD, 128, HID)
        .transpose(1, 0, 2)
        .reshape(128, KD * HID)
    )
    w2p = np.ascontiguousarray(
        (scale * np.asarray(W2, F32))
        .astype(F32)
        .reshape(KH, 128, D)
        .transpose(1, 0, 2)
        .reshape(128, KH * D)
    )
    ident = np.eye(128, dtype=F32)
    ones = np.ones((1, B), F32)
    in_maps = []
    for core in range(NCORES):
        shard = z0[core * B : (core + 1) * B]
        in_maps.append(
            {
                "zt_in": _pack_zT(shard),
                "w1": w1p,
                "w2": w2p,
                "biases": biases,
                "ident": ident,
                "ones": ones,
            }
        )
    return in_maps, c


def run(z0, t, W1, b1, wt, W2, b2, trace=False, mm_dtype=MM_DTYPE, variant=VARIANT):
    from concourse.bass_utils import run_bass_kernel_spmd

    if variant == "b":
        in_maps, c = _make_in_maps_b(z0, t, W1, b1, wt, W2, b2)
        nc = _build_program_b()
    else:
        in_maps, c = _make_in_maps_a(z0, t, W1, b1, wt, W2, b2)
        nc = _build_program_a(mm_dtype=mm_dtype)
    res = run_bass_kernel_spmd(nc, in_maps, core_ids=list(range(NCORES)), trace=trace)

    outs = []
    for core in range(NCORES):
        z_shard = _unpack_zT(np.asarray(res.results[core]["zt_out"], F32))
        outs.append(z_shard)
    out = np.concatenate(outs, axis=0).astype(F32)
    out = out + (NSTEP * c)[None, :].astype(F32)
    return out.astype(F32), res


def kernel(z0, t, W1, b1, wt, W2, b2):
    out, _ = run(z0, t, W1, b1, wt, W2, b2, trace=False)
    return out
